# revision 54
# baseline (speedup 1.0000x reference)
"""Trainium2 Bass kernel for a causal self-attention block (GQA + per-head
RMS-norm + RoPE + learned q-gain), sharded over 8 NeuronCores.

Sharding: data-parallel over batch (B=2) x tensor-parallel over KV groups
(4 groups of 4 query heads). core = b*4 + g. Each core computes attention for
its 4 heads and a partial output projection (its 256 in-dims of Wproj); the
host sums the 4 partials per batch element.

v10: PE-density rewrite. TRN2's PE p-state ramps (0.65 -> 1.2 -> 2.4 GHz
after 3us of back-to-back matmuls), so the whole program is scheduled to
keep the in-order matmul queue dense:
  - chunk-0 QKV runs as half-contraction passes (c=0-3 while c=4-7 is still
    in flight), so the PE starts ~4us after the framework preamble; QKV
    chunks, transposes and output-projection tiles are woven INTO the
    attention tile loops as fillers.
  - each pair's last two PV flushes AND its normalization epilogue are
    deferred into the next pair/block's tile loop (carry), so the drain's
    exp latency and the epilogue's DVE chain are covered by the next pair's
    score matmuls instead of stalling the PE at every boundary. yp is
    allocated lazily at the first PV so the single-bank psy ring orders
    correctly around the deferred writes.
  - causal-mask accumulate matmuls stream only 128 columns (tri is zero
    beyond the diagonal square).
  - PV uses two stationary operands: v_sb (ones col 0 -> even head's
    denominator in PSUM row 0, v in cols 64-127) and v2_sb (v in cols 0-63,
    ones col 64 -> odd head's denominator in row 64). After PV, yp is
    evacuated to SBUF split across vector+scalar so the PSUM bank frees in
    ~0.6us; the denominators are broadcast with ONE selector matmul on the
    PE (no DMA bounce), reciprocal'd full-height on the DVE
    (reciprocal_approx_fast corrupts at a non-zero partition base, so it
    must run base-0), and the normalizing multiplies write y_sb's
    projection layout directly (odd head rows 0-63, even head rows 64-127;
    wp2 is host-packed to match).
  - cos/sin tables are stored once (not per-slab) and broadcast via
    stride-0 free-dim 4D views; RMS-square work is split vector/gpsimd so
    the front-phase u_post chains don't serialize behind either queue.
"""

import math

import numpy as np

import concourse.bacc as bacc
import concourse.bass as bass
import concourse.tile as tile
from concourse import mybir
from concourse.bass import ts
from concourse.bass_utils import run_bass_kernel_spmd
from concourse.masks import make_identity

# Problem dims (hardcoded per contract).
B, S, D, H, KV, HD = 2, 2048, 1024, 16, 4, 64
NH = H // KV          # 4 query heads per core (one KV group)
GD = NH * HD          # 256 out-dims of Wq per group
P = 128               # partitions
NST = S // P          # 16 sequence tiles
JW = 512              # query-block width for attention
NJ = S // JW          # 4 query blocks
NC = 8                # cores
ROPE_BASE = 10000.0
RMS_EPS = 1.1920929e-07
F32 = mybir.dt.float32
F32R = mybir.dt.float32r
F32R = mybir.dt.float32r
BF16 = mybir.dt.bfloat16
I32 = mybir.dt.int32
AXX = mybir.AxisListType.X
ACT = mybir.ActivationFunctionType
ALU = mybir.AluOpType
NQKV = GD + 2 * HD    # 384


def _build_program(reps=1):
    nc = bacc.Bacc("TRN2", target_bir_lowering=False, debug=False)

    xT = nc.dram_tensor("xT", [D, S], BF16, kind="ExternalInput").ap()
    wqkv = nc.dram_tensor("wqkv", [D, NQKV], BF16, kind="ExternalInput").ap()
    wp2 = nc.dram_tensor("wp2", [P, 2 * D], BF16, kind="ExternalInput").ap()
    cos1 = nc.dram_tensor("cos1", [P, NST * HD], F32, kind="ExternalInput").ap()
    sin1 = nc.dram_tensor("sin1", [P, NST * 32], F32, kind="ExternalInput").ap()
    tri = nc.dram_tensor("tri", [P, P], BF16, kind="ExternalInput").ap()
    qg8 = nc.dram_tensor("qg8", [1, NH], F32, kind="ExternalInput").ap()
    ypt = nc.dram_tensor("ypt", [D, S], BF16, kind="ExternalOutput").ap()
    dnb = nc.dram_tensor("dnb", [8, 2 * JW], F32, kind="Internal").ap()

    with tile.TileContext(nc) as tc:
        for _ in range(reps):
            _body(tc, xT, wqkv, wp2, cos1, sin1, tri, qg8, ypt, dnb)
    nc.compile()
    return nc


def _body(tc, xT, wqkv, wp2, cos1, sin1, tri, qg8, ypt, dnb):
    nc = tc.nc
    xTr = xT.rearrange("(c p) s -> p c s", p=P)

    with (
        tc.tile_pool(name="consts", bufs=1) as consts,
        tc.tile_pool(name="xtp", bufs=3) as xtp,
        tc.tile_pool(name="wk", bufs=3) as wk,
        tc.tile_pool(name="rwk", bufs=4) as rwk,
        tc.tile_pool(name="pwk", bufs=3) as pwk,
        tc.tile_pool(name="nwk", bufs=4) as nwk,
        tc.tile_pool(name="psmisc", bufs=2, space="PSUM") as psmisc,
        tc.tile_pool(name="psst", bufs=2, space="PSUM") as psst,
        tc.tile_pool(name="psy", bufs=1, space="PSUM") as psy,
    ):
        # ---------------- persistent SBUF state ----------------
        w_sb = consts.tile([P, 8, NQKV], BF16, name="w_sb")
        wp_sb = consts.tile([P, 2, D], BF16, name="wp_sb")
        cos_sb = consts.tile([P, NST, HD], F32, name="cos_sb")
        sin_sb = consts.tile([P, NST, 32], F32, name="sin_sb")
        tri_sb = consts.tile([P, P], BF16, name="tri_sb")
        qg8_sb = consts.tile([P, NH], F32, name="qg8_sb")
        ident = consts.tile([P, P], BF16, name="ident")
        negI = consts.tile([P, P], BF16, name="negI")
        qT2 = consts.tile([P, 2, S], BF16, name="qT2")
        kTe = consts.tile([P, S], BF16, name="kTe")
        kTo = consts.tile([P, S], BF16, name="kTo")
        # PV stationary operand: col 0 = ones (softmax denominator -> PSUM row
        # 0, where the custom recip/broadcast ops are legal), cols 1-63 = zero,
        # cols 64-127 = v dims (y lands at rows 64-127, 32-aligned).
        # oem is the stationary selector for the PE denominator
        # broadcast: oem[0, r] = 1 for r >= 64 (d_e from den row 0 to out
        # rows 64-127), oem[64, r] = 1 for r < 64 (d_o from den row 64 to
        # out rows 0-63); zero elsewhere. den is pre-zeroed so the full
        # 128-row stationary contraction reads only finite values.
        oem = consts.tile([P, P], BF16, name="oem")
        den2 = consts.tile([P, 2, JW], BF16, name="den2")
        v_sb = consts.tile([P, NST, P], BF16, name="v_sb")
        v2_sb = consts.tile([P, NST, P], BF16, name="v2_sb")
        y_sb = consts.tile([P, 2, S], BF16, name="y_sb")
        qkv_sb = consts.tile([P, NST, 5 * HD], F32, name="qkv_sb")
        ss_all = consts.tile([P, NST * 5], F32, name="ss_all")
        r_all = consts.tile([P, NST * 5], F32, name="r_all")

        # x-tile and weight DMAs first so the first QKV matmul starts ASAP;
        # wp (needed only by C0) goes last.
        wqr = wqkv.rearrange("(c p) n -> p c n", p=P)
        dmaq = [nc.sync, nc.scalar, nc.gpsimd]
        xt0 = xtp.tile([P, 8, JW], BF16, name="xt0", tag="xt")
        # low c-halves of x and w first: the first QKV tiles contract
        # c=0-3 while c=4-7 is still in flight.
        nc.sync.dma_start(out=xt0[:, 0:2, :], in_=xTr[:, 0:2, ts(0, JW)])
        nc.scalar.dma_start(out=w_sb[:, 0:4, :], in_=wqr[:, 0:4, :])
        nc.gpsimd.dma_start(out=xt0[:, 2:4, :], in_=xTr[:, 2:4, ts(0, JW)])
        nc.sync.dma_start(out=xt0[:, 4:6, :], in_=xTr[:, 4:6, ts(0, JW)])
        nc.scalar.dma_start(out=xt0[:, 6:8, :], in_=xTr[:, 6:8, ts(0, JW)])
        nc.gpsimd.dma_start(out=w_sb[:, 4:8, :], in_=wqr[:, 4:8, :])
        make_identity(nc, ident)
        nc.vector.tensor_scalar(
            out=negI, in0=ident, scalar1=-1.0, scalar2=None, op0=ALU.mult
        )

        # zero fills via memset on a bitcast view (f32r cannot be memset
        # directly); ones/zeros for the PV operand are plain bf16 memsets.
        nc.gpsimd.memset(kTe[HD:P, :], 0.0)
        nc.gpsimd.memset(kTo[0:HD, :], 0.0)
        nc.gpsimd.memset(oem, 0.0)
        nc.gpsimd.memset(oem[0:1, HD:P], 1.0)
        nc.gpsimd.memset(oem[HD : HD + 1, 0:HD], 1.0)
        nc.gpsimd.memset(den2, 0.0)
        nc.gpsimd.memset(v_sb[:, :, 0:1], 1.0)
        nc.gpsimd.memset(v_sb[:, :, 1:HD], 0.0)
        nc.gpsimd.memset(v2_sb[:, :, HD : HD + 1], 1.0)
        nc.gpsimd.memset(v2_sb[:, :, HD + 1 : P], 0.0)
        nc.sync.dma_start(out=wp_sb, in_=wp2.rearrange("p (c m) -> p c m", c=2))

        # ---------------- pipelined stream ----------------
        # The attention (B) blocks are rate-limited by the scalar engine's
        # exps, so every independent PE work item (QKV matmuls, output
        # projection) is wrapped in a closure and sprinkled INTO the B tile
        # loops ("fillers") to keep the tensor engine dense (HAM stays warm).
        rot_tiles = {}
        xts = {0: xt0}
        _ps0 = {}

        def u_half(il, half):
            # chunk-0 half-contraction: c=0-3 while c=4-7 still loads
            def go():
                if (il, 0) not in _ps0:
                    _ps0[il] = psmisc.tile(
                        [P, NQKV], F32, name=f"qkv0_{il}", tag="mi"
                    )
                    _ps0[(il, 0)] = True
                qkv_ps = _ps0[il]
                for c in range(4 * half, 4 * half + 4):
                    nc.tensor.matmul(
                        qkv_ps,
                        lhsT=xt0[:, c, ts(il, P)],
                        rhs=w_sb[:, c, :],
                        start=(c == 0),
                        stop=(c == 7),
                    )
                if half == 1:
                    u_stage(0, il, qkv_ps)
            return go

        def u_tile(jb, il):
            def go():
                if il == 0 and jb + 1 < 4:
                    nxt = xtp.tile([P, 8, JW], BF16, name=f"xt{jb+1}", tag="xt")
                    dmaq[(jb + 1) % 3].dma_start(
                        out=nxt[:, 0:4, :], in_=xTr[:, 0:4, ts(jb + 1, JW)]
                    )
                    dmaq[(jb + 2) % 3].dma_start(
                        out=nxt[:, 4:8, :], in_=xTr[:, 4:8, ts(jb + 1, JW)]
                    )
                    xts[jb + 1] = nxt
                xt = xts[jb]
                i = 4 * jb + il
                qkv_ps = psmisc.tile([P, NQKV], F32, name=f"qkv{i}", tag="mi")
                for c in range(8):
                    nc.tensor.matmul(
                        qkv_ps,
                        lhsT=xt[:, c, ts(il, P)],
                        rhs=w_sb[:, c, :],
                        start=(c == 0),
                        stop=(c == 7),
                    )
                u_stage(jb, il, qkv_ps)
            return go

        def u_stage(jb, il, qkv_ps):
            # stage q,k (f32) and v (bf16, both PV layouts); RMS stats.
            # chunks 0/1 feed the front-loaded u_post chain -- keep their
            # squares on the vector engine, off the congested gpsimd queue;
            # the v2 replication (gpsimd, SBUF->SBUF since gpsimd can't read
            # PSUM) is not urgent and goes last.
            i = 4 * jb + il
            nc.vector.tensor_copy(qkv_sb[:, i, :], qkv_ps[:, 0 : 5 * HD])
            nc.vector.tensor_copy(v_sb[:, i, HD:P], qkv_ps[:, 5 * HD : NQKV])
            sq = wk.tile([P, 5 * HD], F32, name=f"sq{i}", tag="sq")
            (nc.vector if jb < 2 else nc.gpsimd).tensor_mul(
                sq, qkv_sb[:, i, :], qkv_sb[:, i, :]
            )
            nc.vector.reduce_sum(
                ss_all[:, 5 * i : 5 * i + 5],
                sq.rearrange("p (h d) -> p h d", d=HD),
                axis=AXX,
            )
            nc.gpsimd.tensor_copy(v2_sb[:, i, 0:HD], v_sb[:, i, HD:P])

        def u_post(jb):
            def go():
                # rsqrt via bitcast magic seed + 2 Newton steps (DVE)
                ssc = ss_all[:, 20 * jb : 20 * jb + 20]
                rc = r_all[:, 20 * jb : 20 * jb + 20]
                mm = wk.tile([P, 20], F32, name=f"m{jb}", tag="m")
                nc.vector.tensor_scalar(
                    out=mm, in0=ssc, scalar1=1.0 / HD, scalar2=RMS_EPS,
                    op0=ALU.mult, op1=ALU.add,
                )
                tt = wk.tile([P, 20], F32, name=f"t{jb}", tag="t")
                nc.vector.tensor_scalar(
                    out=tt.bitcast(I32), in0=mm.bitcast(I32),
                    scalar1=1, scalar2=-1,
                    op0=ALU.logical_shift_right, op1=ALU.bitwise_xor,
                )
                nc.vector.tensor_scalar(
                    out=rc.bitcast(I32), in0=tt.bitcast(I32),
                    scalar1=0x5F3759E0, scalar2=None, op0=ALU.add,
                )
                for _ in range(2):
                    nc.vector.tensor_mul(tt, rc, rc)
                    nc.vector.tensor_mul(tt, tt, mm)
                    nc.vector.tensor_scalar(
                        out=tt, in0=tt, scalar1=-0.5, scalar2=1.5,
                        op0=ALU.mult, op1=ALU.add,
                    )
                    nc.vector.tensor_mul(rc, rc, tt)
                rcv = rc.rearrange("p (t h) -> p t h", h=5)
                nc.vector.tensor_mul(
                    rcv[:, :, 0:NH], rcv[:, :, 0:NH],
                    qg8_sb[:, None, :].broadcast_to([P, 4, NH]),
                )
                # RoPE, mostly on the Pool engine (it is otherwise idle; the
                # DVE is loaded with masks/normalization/copies)
                qc = qkv_sb[:, 4 * jb : 4 * jb + 4, :]
                qcv = qc.rearrange("p t (h d) -> p (t h) d", d=HD)
                qks = rwk.tile([P, 4, 5 * HD], F32, name=f"qks{jb}", tag="qks")
                qksv = qks.rearrange("p t (h d) -> p (t h) d", d=HD)
                nc.vector.tensor_mul(
                    qksv, qcv,
                    rcv.rearrange("p t h -> p (t h)")[:, :, None].broadcast_to([P, 20, HD]),
                )
                rot = rwk.tile([P, 4, 5 * HD], BF16, name=f"rot{jb}", tag="rot")
                qks4 = qks.rearrange("p t (h d) -> p t h d", d=HD)
                rot4 = rot.rearrange("p t (h d) -> p t h d", d=HD)
                cosb = cos_sb[:, 4 * jb : 4 * jb + 4, None, :].broadcast_to(
                    [P, 4, 5, HD]
                )
                nc.vector.tensor_mul(rot4, qks4, cosb)
                rotv = rot.rearrange("p t (h d) -> p (t h) d", d=HD)
                sinb = sin_sb[:, 4 * jb : 4 * jb + 4, None, :].broadcast_to(
                    [P, 4, 5, 32]
                )
                m2a = rwk.tile([P, 20, 32], BF16, name=f"m2a{jb}", tag="m2a")
                nc.vector.tensor_mul(
                    m2a.rearrange("p (t h) d -> p t h d", h=5),
                    qks4[:, :, :, 32:HD], sinb,
                )
                m2b = rwk.tile([P, 20, 32], BF16, name=f"m2b{jb}", tag="m2b")
                nc.vector.tensor_mul(
                    m2b.rearrange("p (t h) d -> p t h d", h=5),
                    qks4[:, :, :, 0:32], sinb,
                )
                nc.vector.tensor_add(rotv[:, :, 0:32], rotv[:, :, 0:32], m2a)
                nc.vector.tensor_sub(rotv[:, :, 32:HD], rotv[:, :, 32:HD], m2b)
                rot_tiles[jb] = rot
            return go

        def tr_chunk(jb):
            rot = rot_tiles[jb]
            for il in range(4):
                i = 4 * jb + il
                for pair in range(2):
                    trp = psmisc.tile([P, P], BF16, name=f"tr{i}_{pair}", tag="mi")
                    nc.tensor.transpose(trp, rot[:, il, ts(pair, P)], ident)
                    nc.vector.tensor_copy(qT2[:, pair, ts(i, P)], trp)
                trk = psmisc.tile([HD, P], BF16, name=f"trk{i}", tag="mi")
                nc.tensor.transpose(trk, rot[:, il, 4 * HD : 5 * HD], ident)
                nc.vector.tensor_copy(kTe[0:HD, ts(i, P)], trk)
            nc.gpsimd.dma_start(
                out=kTo[HD:P, 4 * jb * P : (4 * jb + 4) * P],
                in_=kTe[0:HD, 4 * jb * P : (4 * jb + 4) * P],
            )

        def c_tile(j, mtile):
            def go():
                op = psmisc.tile([P, JW], F32, name=f"op{j}_{mtile}", tag="mi")
                for c in range(2):
                    nc.tensor.matmul(
                        op,
                        lhsT=wp_sb[:, c, ts(mtile, P)],
                        rhs=y_sb[:, c, ts(j, JW)],
                        start=(c == 0),
                        stop=(c == 1),
                    )
                o_sb = nwk.tile([P, JW], BF16, name=f"o{j}_{mtile}", tag="o")
                if j == 3:
                    nc.vector.tensor_copy(o_sb[:, 0 : JW // 2], op[:, 0 : JW // 2])
                    nc.scalar.copy(o_sb[:, JW // 2 : JW], op[:, JW // 2 : JW])
                    [nc.sync, nc.scalar][mtile % 2].dma_start(
                        out=ypt[ts(mtile, P), 3 * JW : 3 * JW + JW // 2],
                        in_=o_sb[:, 0 : JW // 2],
                    )
                    [nc.gpsimd, nc.sync][mtile % 2].dma_start(
                        out=ypt[ts(mtile, P), 3 * JW + JW // 2 : S],
                        in_=o_sb[:, JW // 2 : JW],
                    )
                else:
                    if mtile % 2 == 0:
                        nc.vector.tensor_copy(o_sb, op)
                    else:
                        nc.scalar.copy(o_sb, op)
                    [nc.sync, nc.scalar, nc.gpsimd][mtile % 3].dma_start(
                        out=ypt[ts(mtile, P), ts(j, JW)], in_=o_sb
                    )
            return go

        def b_block(j, fillers, carry_in=None):
            """Attention for q-block j.

            Each tile iteration consumes one deferred action (carry) or one
            filler. Deferred actions are the previous pair's last two PV
            flushes and its normalization epilogue -- running them inside
            the NEXT pair's tile loop lets the next pair's score matmuls
            cover the exp latency of the drain, instead of stalling the
            in-order PE queue at every pair boundary. Returns the final
            pair's deferred actions for the caller to weave onward.
            """
            nt = 4 * (j + 1)
            carry = list(carry_in) if carry_in else []
            for pair in range(2):
                yp = psy.tile([P, 2, JW], F32, name=f"y{j}_{pair}", tag="y")
                pend = []

                def pv_pair(e, yp=yp, nt=nt):
                    pt, pw_, pc0 = e
                    nc.tensor.matmul(
                        yp[:, 0, pc0:JW], lhsT=v_sb[:, pt, :], rhs=pw_[:, 0, pc0:JW],
                        start=(pt == 0), stop=(pt == nt - 1),
                    )
                    nc.tensor.matmul(
                        yp[:, 1, pc0:JW], lhsT=v2_sb[:, pt, :], rhs=pw_[:, 1, pc0:JW],
                        start=(pt == 0), stop=(pt == nt - 1),
                    )

                for t in range(nt):
                    m = t - 4 * j
                    w = JW if m < 0 else JW - P * m
                    c0 = JW - w
                    st = psst.tile([P, 2, JW], F32, name=f"st{j}_{pair}_{t}", tag="st")
                    p_sb = pwk.tile([P, 2, JW], BF16, name=f"p{j}_{pair}_{t}", tag="p")
                    qe = qT2[:, pair, ts(j, JW)]
                    diag = m >= 0
                    nc.tensor.matmul(
                        st[:, 0, c0:JW], lhsT=kTe[:, ts(t, P)], rhs=qe[:, c0:JW],
                        start=True, stop=not diag, skip_group_check=diag,
                    )
                    nc.tensor.matmul(
                        st[:, 1, c0:JW], lhsT=kTo[:, ts(t, P)], rhs=qe[:, c0:JW],
                        start=True, stop=not diag, skip_group_check=diag,
                    )
                    if diag:
                        # tri is zero beyond the 128-col diagonal square, so
                        # the accumulate matmul only streams 128 columns.
                        nc.tensor.matmul(
                            st[:, 0, c0 : c0 + P], lhsT=negI, rhs=tri_sb[:, 0:P],
                            start=False, stop=True, skip_group_check=True,
                        )
                        nc.tensor.matmul(
                            st[:, 1, c0 : c0 + P], lhsT=negI, rhs=tri_sb[:, 0:P],
                            start=False, stop=True, skip_group_check=True,
                        )
                    if carry:
                        g = carry.pop(0)
                        if g is None:
                            if fillers:
                                fillers.pop(0)()
                        else:
                            g()
                    elif fillers:
                        fillers.pop(0)()
                    if len(pend) >= 2:
                        pv_pair(pend.pop(0))
                    nc.scalar.activation(p_sb[:, :, c0:JW], st[:, :, c0:JW], ACT.Exp)
                    pend.append((t, p_sb, c0))
                while pend:
                    pv_pair(pend.pop(0))
                rem = []
                # evacuate yp to SBUF (split across vector + scalar so the
                # PSUM bank frees early); the copies semaphore-wait on the
                # deferred PVs. The last pair reads straight from PSUM (no
                # later PV reuses its bank).
                if j == 3 and pair == 1:
                    ycp = yp
                    nc.vector.tensor_copy(den2[0:1, pair, :], yp[0:1, 0, :])
                    nc.scalar.copy(den2[HD : HD + 1, pair, :], yp[HD : HD + 1, 1, :])
                else:
                    ycp = nwk.tile([P, 2, JW], F32, name=f"ycp{j}_{pair}", tag="ycp")
                    nc.vector.tensor_copy(ycp[:, 0, :], yp[:, 0, :])
                    nc.scalar.copy(ycp[:, 1, :], yp[:, 1, :])
                    nc.vector.tensor_copy(den2[0:1, pair, :], ycp[0:1, 0, :])
                    nc.scalar.copy(den2[HD : HD + 1, pair, :], ycp[HD : HD + 1, 1, :])

                def fin(pair=pair, ycp=ycp):
                    # PE broadcast of the denominators (d_o -> rows 0-63,
                    # d_e -> rows 64-127), reciprocal, then multiply
                    # straight into y_sb's projection layout.
                    bcp = psmisc.tile([P, JW], F32, name=f"bc{j}_{pair}", tag="mi")
                    nc.tensor.matmul(bcp, lhsT=oem, rhs=den2[:, pair, :])
                    rcp = nwk.tile([P, JW], F32, name=f"rc{j}_{pair}", tag="rcp")
                    nc.vector.reciprocal_approx_fast(rcp, bcp)
                    nc.vector.tensor_mul(
                        y_sb[0:HD, pair, ts(j, JW)], ycp[0:HD, 1, :], rcp[0:HD, :]
                    )
                    nc.vector.tensor_mul(
                        y_sb[HD:P, pair, ts(j, JW)], ycp[HD:P, 0, :], rcp[HD:P, :]
                    )

                if pair == 0:
                    carry = rem + [None, fin] if not rem else rem + [fin]
                else:
                    for g in rem:
                        g()
                    fin()
            while fillers:
                fillers.pop(0)()
            return []

        # program order: QKV chunks, transposes and projection tiles are
        # woven into the attention blocks as fillers so the PE queue stays
        # dense (p-state remains at 2.4 GHz); each block also consumes the
        # previous block's deferred PV flushes + normalization epilogue.
        xt1 = xtp.tile([P, 8, JW], BF16, name="xt1", tag="xt")
        nc.sync.dma_start(out=xt1[:, 0:4, :], in_=xTr[:, 0:4, ts(1, JW)])
        nc.scalar.dma_start(out=xt1[:, 4:8, :], in_=xTr[:, 4:8, ts(1, JW)])
        xts[1] = xt1
        nc.gpsimd.dma_start(
            out=cos_sb, in_=cos1.rearrange("p (t f) -> p t f", t=NST)
        )
        nc.gpsimd.dma_start(
            out=sin_sb, in_=sin1.rearrange("p (t f) -> p t f", t=NST)
        )
        nc.scalar.dma_start(out=tri_sb, in_=tri)
        nc.gpsimd.dma_start(out=qg8_sb, in_=qg8.to_broadcast([P, NH]))
        u_half(0, 0)()
        u_half(1, 0)()
        u_half(0, 1)()
        u_half(1, 1)()
        u_half(2, 0)()
        u_half(2, 1)()
        u_half(3, 0)()
        u_half(3, 1)()
        make_identity(nc, ident)
        nc.vector.tensor_scalar(
            out=negI, in0=ident, scalar1=-1.0, scalar2=None, op0=ALU.mult
        )
        u_post(0)()
        for il in range(4):
            u_tile(1, il)()
        tr_chunk(0)
        u_post(1)()
        cr = b_block(
            0, [u_tile(2, il) for il in range(4)] + [u_tile(3, 0), u_tile(3, 1)]
        )
        tr_chunk(1)
        cr = b_block(
            1,
            [u_post(2), u_tile(3, 2), u_tile(3, 3)]
            + [c_tile(0, mt) for mt in range(8)],
            carry_in=cr,
        )
        tr_chunk(2)
        cr = b_block(
            2, [u_post(3)] + [c_tile(1, mt) for mt in range(8)], carry_in=cr
        )
        tr_chunk(3)
        cr = b_block(3, [c_tile(2, mt) for mt in range(8)], carry_in=cr)
        for f in cr:
            f()
        for mt in range(8):
            c_tile(3, mt)()
